# revision 11
# baseline (speedup 1.0000x reference)
"""Causal self-attention Bass/Tile kernel for 8 Trainium2 NeuronCores.

Problem (hardcoded): x (4, 2048, 1024) f32, w_attn (1024, 3072), w_proj
(1024, 1024).  H=16 heads, D=64.  Output: (4, 2048, 1024) f32.

Sharding: core c handles batch b = c // 2 and head-group hg = c % 2
(8 heads each).  Data parallel on B, tensor parallel on heads: each core
gets the w_attn columns for its heads (q|k|v, each 512 cols) and the
w_proj rows for its heads (512 rows).  Per-core output is a partial sum
over head groups; the host adds the two partials per batch.

All matmul operands are fp16 (host pre-converts x and the weights): same
1 cyc/row PE speed as float32r but half the SBUF footprint, no N>=256
fast-path restriction (so diagonal tiles compute only their valid span),
and eligibility for the DMA xbar-transpose path.  PSUM accumulation stays
fp32.  Measured end-to-end rel err: ~4e-4.

Per-core structure (strips of 512 queries):
  phase 1 (per strip): x^T tiles arrive directly via transpose-DMA (no
           PE transposes).  Q^T/K^T ([d, tok], head pairs stacked on
           partitions) accumulate over 8 e-chunks; V is written into
           vaug tiles [128 keys, 8*(64 V | 64 ones)] -- the 64 replicated
           ones columns make the exp@V matmul emit each head's softmax
           row sums pre-broadcast across PSUM partitions 64:128.
  phase 2: per head-pair, per key-tile t: scores^T = K^T.T @ Q^T (two
           K=64 matmuls on disjoint PE row groups), ONE fused exp over
           both heads' scores via a 3D AP on a 2-bank [128,1024] PSUM
           tile (scale 1/sqrt(64) folded in), causal masking of the
           diagonal band via one 3D gpsimd affine_select, then per-head
           [128,128] x [128,512-c0] matmuls accumulate exp@V into PSUM
           (y on partitions 0:64, sums broadcast on 64:128).  Columns
           below the causal boundary of diagonal tiles are skipped
           exactly (c0 = 128*dshift).
           Pair tail: rec = 1/sums via a partition-shifted DVE
           reciprocal (PSUM rows 64:128 -> SBUF rows 0:64), then two
           fused multiply-copies produce normalized y^T fp16 directly.
           No DRAM bounce, no deferred normalization.
  phase 3: out partial = y^T.T @ w_proj over 8 output tiles.

  Pipelining: phase-1 work of strip s+1 and phase-3 work of older strips
  are drip-fed between the t-loop iterations of strip s's attention so
  the PE always has independent fill work while ACT runs the exp stream.
  Projections are deferred to the latest ACT-paced strips: p3(0) fills
  phase2(2), p3(1)+p3(2) fill phase2(3), p3(3) runs in the tail.

PSUM budget (8 banks): ps (scores, [128,1024] = 2 banks) x2, ph1
(qkv/v/proj) x2, py (exp@V accumulators, one per head) x2.
"""

import os
from contextlib import ExitStack

import numpy as np

import concourse.bass as bass
import concourse.bacc as bacc
import concourse.mybir as mybir
import concourse.tile as tile
from concourse.bass_utils import run_bass_kernel_spmd

F32 = mybir.dt.float32
F16 = mybir.dt.float16
EXP = mybir.ActivationFunctionType.Exp

S = 2048          # sequence length
E = 1024          # embedding
D = 64            # head dim
HL = 8            # heads per core
NP = 4            # head pairs per core
EC = 8            # E / 128 chunks
NSTRIP = 4        # query strips of 512
TPS = 4           # 128-token tiles per strip
NT = 16           # 128-key tiles total


def emit_kernel(ctx, tc, out, x, w_qkv, w_proj):
    nc = tc.nc

    wpool = ctx.enter_context(tc.tile_pool(name="weights", bufs=1))
    kv = ctx.enter_context(tc.tile_pool(name="kv", bufs=1))
    work = ctx.enter_context(tc.tile_pool(name="work", bufs=1))
    psum = ctx.enter_context(tc.tile_pool(name="psum", bufs=1, space="PSUM"))

    # ---- resident weights (DRAM already fp16, host-converted) ----
    # DMA order matters for startup: strip-0 x^T transposes are issued
    # first (in whole_body), then wqk (gates the first qk units), then wv;
    # wpj goes on the ACT hwdge queue (idle at start, needed only late).
    wqk = [wpool.tile([128, 1024], F16, name=f"wqk{e}", tag=f"wqk{e}")
           for e in range(EC)]
    wv = [wpool.tile([128, 512], F16, name=f"wv{e}", tag=f"wv{e}")
          for e in range(EC)]
    wpj = [wpool.tile([128, 1024], F16, name=f"wpj{f}", tag=f"wpj{f}")
           for f in range(NP)]

    def load_weights():
        # wv on the ACT hwdge queue (parallel with the sync queue's x^T
        # transposes), wqk on sync behind the transposes, wpj last.
        for e in range(EC):
            nc.scalar.dma_start(out=wv[e][:],
                                in_=w_qkv[e * 128:(e + 1) * 128, 1024:1536])
        for e in range(EC):
            nc.sync.dma_start(out=wqk[e][:],
                              in_=w_qkv[e * 128:(e + 1) * 128, 0:1024])
        for f in range(NP):
            nc.scalar.dma_start(out=wpj[f][:],
                                in_=w_proj[f * 128:(f + 1) * 128, :])

    # ---- persistent K^T (pair-stacked) and V||ones (8 heads x 128) ----
    kT = [kv.tile([128, S], F16, name=f"kT{p}", tag=f"kT{p}")
          for p in range(NP)]
    vaug = [kv.tile([128, 1024], F16, name=f"vaug_{t}", tag=f"vaug_{t}")
            for t in range(NT)]

    state = {}

    def load_xT(s):
        """Issue the 8 transpose-DMAs for strip s's x^T tiles."""
        xT = [work.tile([128, 512], F16, name=f"xT{e}_{s}", tag=f"xT{e}",
                        bufs=2) for e in range(EC)]
        state[("xT", s)] = xT
        r0 = s * 512
        for e in range(EC):
            nc.sync.dma_start_transpose(
                xT[e][:], x[r0:r0 + 512, e * 128:(e + 1) * 128])

    def qk_unit(s, p, which):
        """Q^T (or K^T) for pair p of strip s: 8 matmuls + copy."""
        xT = state[("xT", s)]
        if ("qT", s) not in state:
            state[("qT", s)] = [
                work.tile([128, 512], F16, name=f"qT{p}_{s}",
                          tag=f"qT{p}", bufs=2)
                for p in range(NP)]
        qT = state[("qT", s)]
        co = (0 if which == "q" else 512) + p * 128
        pqk = psum.tile([128, 512], F32, name=f"p{which}_{s}_{p}",
                        tag="ph1", bufs=2)
        for e in range(EC):
            nc.tensor.matmul(pqk[:], wqk[e][:, co:co + 128], xT[e][:],
                             start=(e == 0), stop=(e == EC - 1))
        if which == "q":
            nc.vector.tensor_copy(qT[p][:], pqk[:])
        else:
            nc.vector.tensor_copy(kT[p][:, s * 512:(s + 1) * 512], pqk[:])

    def v_unit(s, tt):
        """V||ones for x tile tt of strip s: 8 matmuls + ones + copy."""
        xT = state[("xT", s)]
        pv = psum.tile([128, 512], F32, name=f"pv_{s}_{tt}", tag="ph1",
                       bufs=2)
        for e in range(EC):
            nc.tensor.matmul(pv[:], xT[e][:, tt * 128:(tt + 1) * 128],
                             wv[e][:], start=(e == 0), stop=(e == EC - 1))
        va = vaug[s * TPS + tt]
        va3 = va.rearrange("p (h c) -> p h c", c=128)
        nc.gpsimd.memset(va3[:, :, 64:128], 1.0)
        nc.vector.tensor_copy(va3[:, :, 0:64],
                              pv[:].rearrange("p (h c) -> p h c", c=64))

    def p3_unit(s, tt, eo):
        """Projection for strip s, output tile (tt, eo)."""
        yT = state[("yT", s)]
        po = psum.tile([128, 512], F32, name=f"po_{s}_{tt}_{eo}",
                       tag="ph1", bufs=2)
        for p in range(NP):
            nc.tensor.matmul(
                po[:], yT[p][:, tt * 128:(tt + 1) * 128],
                wpj[p][:, eo * 512:(eo + 1) * 512],
                start=(p == 0), stop=(p == NP - 1))
        osb = work.tile([128, 512], F32, name=f"osb_{s}_{tt}_{eo}",
                        tag="osb", bufs=4)
        nc.vector.tensor_copy(osb[:], po[:])
        r0 = (s * TPS + tt) * 128
        nc.sync.dma_start(
            out=out[r0:r0 + 128, eo * 512:(eo + 1) * 512], in_=osb[:])

    def qk_units(s, pairs):
        return [lambda s=s, p=p, w=w: qk_unit(s, p, w)
                for p in pairs for w in ("q", "k")]

    def v_units(s):
        return [lambda s=s, tt=tt: v_unit(s, tt) for tt in range(TPS)]

    def p3_units(s):
        return [lambda s=s, tt=tt, eo=eo: p3_unit(s, tt, eo)
                for tt in range(TPS) for eo in range(2)]

    def phase2(s, units):
        """Attention for strip s.  `units` are independent emission closures
        drip-fed into the t-loop so the PE always has fill work while ACT
        paces the exp stream."""
        qT = state[("qT", s)]
        state[("yT", s)] = [
            work.tile([128, 512], F16, name=f"yT{p}_{s}", tag=f"yT{p}",
                      bufs=3)
            for p in range(NP)]
        ntile = 4 * s + 4
        units = list(units)
        nslots = NP * ntile
        rate = len(units) / nslots
        pulled = 0
        slot = 0

        def pull():
            nonlocal pulled, slot
            slot += 1
            while pulled < len(units) and pulled < rate * slot:
                units[pulled]()
                pulled += 1

        for p in range(NP):
            py_a = psum.tile([128, 512], F32, name=f"pya_{s}_{p}", tag="py",
                             bufs=2)
            py_b = psum.tile([128, 512], F32, name=f"pyb_{s}_{p}", tag="py",
                             bufs=2)

            def scores_exp(t):
                # diagonal tiles: columns below 128*dshift are fully masked
                dshift = t - 4 * s
                c0 = 0 if dshift < 0 else 128 * dshift
                ksl = kT[p][:, t * 128:(t + 1) * 128]
                ps_a = psum.tile([128, 512], F32, name=f"psa_{s}_{p}_{t}",
                                 tag="ps", bufs=4)
                ps_b = psum.tile([128, 512], F32, name=f"psb_{s}_{p}_{t}",
                                 tag="ps", bufs=4)
                nc.tensor.matmul(ps_a[:, c0:], ksl[0:64, :],
                                 qT[p][0:64, c0:], start=True, stop=True)
                nc.tensor.matmul(ps_b[:, c0:], ksl[64:128, :],
                                 qT[p][64:128, c0:], start=True, stop=True,
                                 tile_position=(64, 0))
                es = work.tile([128, 1024], F16, name=f"es_{s}_{p}_{t}",
                               tag="es", bufs=3)
                # two plain 2D activations (a fused 3D-AP activation is ~3x
                # slower on HW: strided reads break ACT streaming)
                nc.scalar.activation(es[:, c0:512], ps_a[:, c0:], EXP,
                                     scale=0.125)
                nc.scalar.activation(es[:, 512 + c0:1024],
                                     ps_b[:, c0:], EXP, scale=0.125)
                if dshift >= 0:  # causal mask on the partially-valid band
                    for h in (0, 1):
                        sl = slice(512 * h + c0, 512 * h + c0 + 128)
                        nc.gpsimd.affine_select(
                            out=es[:, sl], in_=es[:, sl],
                            compare_op=mybir.AluOpType.is_ge, fill=0.0,
                            base=0, channel_multiplier=-1,
                            pattern=[[1, 128]])
                return es, c0

            def av_sums(t, es, c0):
                st = (t == 0)
                sp = (t == ntile - 1)
                vA = vaug[t][:, (2 * p) * 128:(2 * p) * 128 + 128]
                vB = vaug[t][:, (2 * p + 1) * 128:(2 * p + 1) * 128 + 128]
                nc.tensor.matmul(py_a[:, c0:], vA, es[:, c0:512],
                                 start=st, stop=sp)
                nc.tensor.matmul(py_b[:, c0:], vB, es[:, 512 + c0:1024],
                                 start=st, stop=sp)

            # software pipeline: issue scores(t+1) before exp@V(t) so the
            # PE never waits on ACT's exp; drip filler units in per slot.
            prev = scores_exp(0)
            for t in range(1, ntile):
                cur = scores_exp(t)
                av_sums(t - 1, *prev)
                pull()
                prev = cur
            av_sums(ntile - 1, *prev)
            pull()
            del prev

            # pair tail: rec = 1/sums (partition-shifted from PSUM rows
            # 64:128), then normalized y^T via fused multiply-copies.
            yT = state[("yT", s)]
            rec_a = work.tile([64, 512], F32, name=f"reca_{s}_{p}",
                              tag="rec", bufs=2)
            rec_b = work.tile([64, 512], F32, name=f"recb_{s}_{p}",
                              tag="rec", bufs=2)
            nc.vector.reciprocal(rec_a[:], py_a[64:128, :])
            nc.vector.reciprocal(rec_b[:], py_b[64:128, :])
            nc.vector.tensor_mul(yT[p][0:64, :], py_a[0:64, :], rec_a[:])
            nc.vector.tensor_mul(yT[p][64:128, :], py_b[0:64, :], rec_b[:])
        while pulled < len(units):
            units[pulled]()
            pulled += 1

    # weights are loaded once per kernel invocation, outside the timing
    # repeat loop (matches the baseline measurement methodology)
    load_weights()

    def whole_body():
        state.clear()
        load_xT(0)
        # minimal phase-1 prefix for pair 0's attention; v units first
        # (wv arrives on the ACT queue before wqk finishes on sync), the
        # rest of strip 0's qk units drip into phase2(0) as filler.
        for u in v_units(0) + qk_units(0, [0]):
            u()
        for s in range(NSTRIP):
            units = []
            if s == 0:
                units.extend(qk_units(0, [1, 2, 3]))
            if s + 1 < NSTRIP:
                load_xT(s + 1)
                units.extend(qk_units(s + 1, range(NP)))
                units.extend(v_units(s + 1))
            if s == 2:
                units.extend(p3_units(0))
            if s == 3:
                units.extend(p3_units(1))
                units.extend(p3_units(2))
            phase2(s, units)
        for u in p3_units(NSTRIP - 1):
            u()

    repeat = int(os.environ.get("KREPEAT", "1"))
    if repeat > 1:
        # timing-only mode: run the whole computation `repeat` times
        # (idempotent) so marginal wall-clock per iteration = HW exec time
        with tc.For_i(0, repeat, 1):
            whole_body()
    else:
        whole_body()


_CACHE = {}


def build_nc():
    if "nc" in _CACHE:
        return _CACHE["nc"]
    nc = bacc.Bacc("TRN2", target_bir_lowering=False, debug=False,
                   enable_asserts=False, num_devices=8)
    x = nc.dram_tensor("x", [S, E], F16, kind="ExternalInput").ap()
    w_qkv = nc.dram_tensor("w_qkv", [E, 1536], F16,
                           kind="ExternalInput").ap()
    w_proj = nc.dram_tensor("w_proj", [512, E], F16,
                            kind="ExternalInput").ap()
    out = nc.dram_tensor("out", [S, E], F32, kind="ExternalOutput").ap()
    with tile.TileContext(nc) as tc:
        with ExitStack() as ctx:
            emit_kernel(ctx, tc, out, x, w_qkv, w_proj)
    nc.compile()
    _CACHE["nc"] = nc
    return nc


def make_in_maps(x, w_attn, w_proj):
    x = np.asarray(x, dtype=np.float32)
    w_attn = np.asarray(w_attn, dtype=np.float32)
    w_proj = np.asarray(w_proj, dtype=np.float32)
    in_maps = []
    for c in range(8):
        b, hg = divmod(c, 2)
        lo, hi = hg * 512, (hg + 1) * 512
        wq = w_attn[:, lo:hi]
        wk = w_attn[:, 1024 + lo:1024 + hi]
        wv = w_attn[:, 2048 + lo:2048 + hi]
        wqkv = np.ascontiguousarray(
            np.concatenate([wq, wk, wv], axis=1)).astype(np.float16)
        wp = np.ascontiguousarray(w_proj[lo:hi, :]).astype(np.float16)
        in_maps.append({
            "x": np.ascontiguousarray(x[b]).astype(np.float16),
            "w_qkv": wqkv,
            "w_proj": wp,
        })
    return in_maps


def gather(results):
    parts = [results[c]["out"] for c in range(8)]
    return np.stack([parts[2 * b] + parts[2 * b + 1] for b in range(4)]).astype(
        np.float32)


def kernel(x, w_attn, w_proj):
    nc = build_nc()
    res = run_bass_kernel_spmd(nc, make_in_maps(x, w_attn, w_proj),
                               core_ids=list(range(8)))
    return gather(res.results)


# revision 12
# speedup vs baseline: 1.0818x; 1.0818x over previous
"""Causal self-attention Bass/Tile kernel for 8 Trainium2 NeuronCores.

Problem (hardcoded): x (4, 2048, 1024) f32, w_attn (1024, 3072), w_proj
(1024, 1024).  H=16 heads, D=64.  Output: (4, 2048, 1024) f32.

Sharding: core c handles batch b = c // 2 and head-group hg = c % 2
(8 heads each).  Data parallel on B, tensor parallel on heads: each core
gets the w_attn columns for its heads (q|k|v, each 512 cols) and the
w_proj rows for its heads (512 rows).  Per-core output is a partial sum
over head groups; the host adds the two partials per batch.

All matmul operands are fp16 (host pre-converts x and the weights): same
1 cyc/row PE speed as float32r but half the SBUF footprint, no N>=256
fast-path restriction (so diagonal tiles compute only their valid span),
and eligibility for the DMA xbar-transpose path.  PSUM accumulation stays
fp32.  Measured end-to-end rel err: ~4e-4.

Per-core structure (strips of 512 queries):
  phase 1 (per strip): x^T tiles arrive directly via transpose-DMA (no
           PE transposes).  Q^T/K^T ([d, tok], head pairs stacked on
           partitions) accumulate over 8 e-chunks; V is written into
           vaug tiles [128 keys, 8*(64 V | 64 ones)] -- the 64 replicated
           ones columns make the exp@V matmul emit each head's softmax
           row sums pre-broadcast across PSUM partitions 64:128.
  phase 2: per head-pair, per key-tile t: scores^T = K^T.T @ Q^T (two
           K=64 matmuls on disjoint PE row groups), ONE fused exp over
           both heads' scores via a 3D AP on a 2-bank [128,1024] PSUM
           tile (scale 1/sqrt(64) folded in), causal masking of the
           diagonal band via one 3D gpsimd affine_select, then per-head
           [128,128] x [128,512-c0] matmuls accumulate exp@V into PSUM
           (y on partitions 0:64, sums broadcast on 64:128).  Columns
           below the causal boundary of diagonal tiles are skipped
           exactly (c0 = 128*dshift).
           Pair tail: rec = 1/sums via a partition-shifted DVE
           reciprocal (PSUM rows 64:128 -> SBUF rows 0:64), then two
           fused multiply-copies produce normalized y^T fp16 directly.
           No DRAM bounce, no deferred normalization.
  phase 3: out partial = y^T.T @ w_proj over 8 output tiles.

  Pipelining: phase-1 work of strip s+1 and phase-3 work of older strips
  are drip-fed between the t-loop iterations of strip s's attention so
  the PE always has independent fill work while ACT runs the exp stream.
  Projections are deferred to the latest ACT-paced strips: p3(0) fills
  phase2(2), p3(1)+p3(2) fill phase2(3), p3(3) runs in the tail.

PSUM budget (8 banks): ps (scores, [128,1024] = 2 banks) x2, ph1
(qkv/v/proj) x2, py (exp@V accumulators, one per head) x2.
"""

import os
from contextlib import ExitStack

import numpy as np

import concourse.bass as bass
import concourse.bacc as bacc
import concourse.mybir as mybir
import concourse.tile as tile
from concourse.bass_utils import run_bass_kernel_spmd

F32 = mybir.dt.float32
F16 = mybir.dt.float16
EXP = mybir.ActivationFunctionType.Exp

S = 2048          # sequence length
E = 1024          # embedding
D = 64            # head dim
HL = 8            # heads per core
NP = 4            # head pairs per core
EC = 8            # E / 128 chunks
NSTRIP = 4        # query strips of 512
TPS = 4           # 128-token tiles per strip
NT = 16           # 128-key tiles total


def emit_kernel(ctx, tc, out, x, w_qkv, w_proj):
    nc = tc.nc

    wpool = ctx.enter_context(tc.tile_pool(name="weights", bufs=1))
    kv = ctx.enter_context(tc.tile_pool(name="kv", bufs=1))
    work = ctx.enter_context(tc.tile_pool(name="work", bufs=1))
    psum = ctx.enter_context(tc.tile_pool(name="psum", bufs=1, space="PSUM"))

    # ---- resident weights (DRAM already fp16, host-converted) ----
    # DMA order matters for startup: strip-0 x^T transposes are issued
    # first (in whole_body), then wqk (gates the first qk units), then wv;
    # wpj goes on the ACT hwdge queue (idle at start, needed only late).
    wqk = [wpool.tile([128, 1024], F16, name=f"wqk{e}", tag=f"wqk{e}")
           for e in range(EC)]
    wv = [wpool.tile([128, 512], F16, name=f"wv{e}", tag=f"wv{e}")
          for e in range(EC)]
    wpj = [wpool.tile([128, 1024], F16, name=f"wpj{f}", tag=f"wpj{f}")
           for f in range(NP)]

    def load_weights():
        # wv on the ACT hwdge queue (parallel with the sync queue's x^T
        # transposes), wqk on sync behind the transposes, wpj last.
        for e in range(EC):
            nc.scalar.dma_start(out=wv[e][:],
                                in_=w_qkv[e * 128:(e + 1) * 128, 1024:1536])
        for e in range(EC):
            nc.sync.dma_start(out=wqk[e][:],
                              in_=w_qkv[e * 128:(e + 1) * 128, 0:1024])
        for f in range(NP):
            nc.scalar.dma_start(out=wpj[f][:],
                                in_=w_proj[f * 128:(f + 1) * 128, :])

    # ---- persistent K^T (pair-stacked) and V||ones (8 heads x 128) ----
    kT = [kv.tile([128, S], F16, name=f"kT{p}", tag=f"kT{p}")
          for p in range(NP)]
    vaug = [kv.tile([128, 1024], F16, name=f"vaug_{t}", tag=f"vaug_{t}")
            for t in range(NT)]

    state = {}

    def load_xT(s):
        """Issue the 8 transpose-DMAs for strip s's x^T tiles."""
        xT = [work.tile([128, 512], F16, name=f"xT{e}_{s}", tag=f"xT{e}",
                        bufs=2) for e in range(EC)]
        state[("xT", s)] = xT
        r0 = s * 512
        for e in range(EC):
            nc.sync.dma_start_transpose(
                xT[e][:], x[r0:r0 + 512, e * 128:(e + 1) * 128])

    def qk_unit(s, p, which):
        """Q^T (or K^T) for pair p of strip s: 8 matmuls + copy."""
        xT = state[("xT", s)]
        if ("qT", s) not in state:
            state[("qT", s)] = [
                work.tile([128, 512], F16, name=f"qT{p}_{s}",
                          tag=f"qT{p}", bufs=2)
                for p in range(NP)]
        qT = state[("qT", s)]
        co = (0 if which == "q" else 512) + p * 128
        pqk = psum.tile([128, 512], F32, name=f"p{which}_{s}_{p}",
                        tag="ph1", bufs=2)
        for e in range(EC):
            nc.tensor.matmul(pqk[:], wqk[e][:, co:co + 128], xT[e][:],
                             start=(e == 0), stop=(e == EC - 1))
        if which == "q":
            nc.vector.tensor_copy(qT[p][:], pqk[:])
        else:
            nc.vector.tensor_copy(kT[p][:, s * 512:(s + 1) * 512], pqk[:])

    def v_unit(s, tt):
        """V||ones for x tile tt of strip s: 8 matmuls + ones + copy."""
        xT = state[("xT", s)]
        pv = psum.tile([128, 512], F32, name=f"pv_{s}_{tt}", tag="ph1",
                       bufs=2)
        for e in range(EC):
            nc.tensor.matmul(pv[:], xT[e][:, tt * 128:(tt + 1) * 128],
                             wv[e][:], start=(e == 0), stop=(e == EC - 1))
        va = vaug[s * TPS + tt]
        va3 = va.rearrange("p (h c) -> p h c", c=128)
        nc.gpsimd.memset(va3[:, :, 64:128], 1.0)
        nc.vector.tensor_copy(va3[:, :, 0:64],
                              pv[:].rearrange("p (h c) -> p h c", c=64))

    def p3_unit(s, tt, eo):
        """Projection for strip s, output tile (tt, eo)."""
        yT = state[("yT", s)]
        po = psum.tile([128, 512], F32, name=f"po_{s}_{tt}_{eo}",
                       tag="ph1", bufs=2)
        for p in range(NP):
            nc.tensor.matmul(
                po[:], yT[p][:, tt * 128:(tt + 1) * 128],
                wpj[p][:, eo * 512:(eo + 1) * 512],
                start=(p == 0), stop=(p == NP - 1))
        osb = work.tile([128, 512], F32, name=f"osb_{s}_{tt}_{eo}",
                        tag="osb", bufs=4)
        nc.vector.tensor_copy(osb[:], po[:])
        r0 = (s * TPS + tt) * 128
        nc.sync.dma_start(
            out=out[r0:r0 + 128, eo * 512:(eo + 1) * 512], in_=osb[:])

    def qk_units(s, pairs):
        return [lambda s=s, p=p, w=w: qk_unit(s, p, w)
                for p in pairs for w in ("q", "k")]

    def v_units(s):
        return [lambda s=s, tt=tt: v_unit(s, tt) for tt in range(TPS)]

    def p3_units(s):
        return [lambda s=s, tt=tt, eo=eo: p3_unit(s, tt, eo)
                for tt in range(TPS) for eo in range(2)]

    def phase2(s, units):
        """Attention for strip s.  `units` are independent emission closures
        drip-fed into the t-loop so the PE always has fill work while ACT
        paces the exp stream."""
        qT = state[("qT", s)]
        state[("yT", s)] = [
            work.tile([128, 512], F16, name=f"yT{p}_{s}", tag=f"yT{p}",
                      bufs=3)
            for p in range(NP)]
        ntile = 4 * s + 4
        units = list(units)
        nslots = NP * ntile
        rate = len(units) / nslots
        pulled = 0
        slot = 0

        def pull():
            nonlocal pulled, slot
            slot += 1
            while pulled < len(units) and pulled < rate * slot:
                units[pulled]()
                pulled += 1

        for p in range(NP):
            py_a = psum.tile([128, 512], F32, name=f"pya_{s}_{p}", tag="py",
                             bufs=2)
            py_b = psum.tile([128, 512], F32, name=f"pyb_{s}_{p}", tag="py",
                             bufs=2)

            def scores_exp(t):
                # diagonal tiles: columns below 128*dshift are fully masked
                dshift = t - 4 * s
                c0 = 0 if dshift < 0 else 128 * dshift
                ksl = kT[p][:, t * 128:(t + 1) * 128]
                ps = psum.tile([128, 1024], F32, name=f"ps_{s}_{p}_{t}",
                               tag="ps", bufs=2)
                nc.tensor.matmul(ps[:, c0:512], ksl[0:64, :],
                                 qT[p][0:64, c0:], start=True, stop=True)
                nc.tensor.matmul(ps[:, 512 + c0:1024], ksl[64:128, :],
                                 qT[p][64:128, c0:], start=True, stop=True,
                                 tile_position=(64, 0))
                es = work.tile([128, 1024], F16, name=f"es_{s}_{p}_{t}",
                               tag="es", bufs=3)
                # two plain 2D activations (a fused 3D-AP activation is ~3x
                # slower on HW: strided reads break ACT streaming)
                nc.scalar.activation(es[:, c0:512], ps[:, c0:512], EXP,
                                     scale=0.125)
                nc.scalar.activation(es[:, 512 + c0:1024],
                                     ps[:, 512 + c0:1024], EXP, scale=0.125)
                if dshift >= 0:  # causal mask on the partially-valid band
                    for h in (0, 1):
                        sl = slice(512 * h + c0, 512 * h + c0 + 128)
                        nc.gpsimd.affine_select(
                            out=es[:, sl], in_=es[:, sl],
                            compare_op=mybir.AluOpType.is_ge, fill=0.0,
                            base=0, channel_multiplier=-1,
                            pattern=[[1, 128]])
                return es, c0

            def av_sums(t, es, c0):
                st = (t == 0)
                sp = (t == ntile - 1)
                vA = vaug[t][:, (2 * p) * 128:(2 * p) * 128 + 128]
                vB = vaug[t][:, (2 * p + 1) * 128:(2 * p + 1) * 128 + 128]
                nc.tensor.matmul(py_a[:, c0:], vA, es[:, c0:512],
                                 start=st, stop=sp)
                nc.tensor.matmul(py_b[:, c0:], vB, es[:, 512 + c0:1024],
                                 start=st, stop=sp)

            # software pipeline: issue scores(t+1) before exp@V(t) so the
            # PE never waits on ACT's exp; drip filler units in per slot.
            prev = scores_exp(0)
            for t in range(1, ntile):
                cur = scores_exp(t)
                av_sums(t - 1, *prev)
                pull()
                prev = cur
            av_sums(ntile - 1, *prev)
            pull()
            del prev

            # pair tail: rec = 1/sums (partition-shifted from PSUM rows
            # 64:128), then normalized y^T via fused multiply-copies.
            yT = state[("yT", s)]
            rec_a = work.tile([64, 512], F32, name=f"reca_{s}_{p}",
                              tag="rec", bufs=2)
            rec_b = work.tile([64, 512], F32, name=f"recb_{s}_{p}",
                              tag="rec", bufs=2)
            nc.vector.reciprocal(rec_a[:], py_a[64:128, :])
            nc.vector.reciprocal(rec_b[:], py_b[64:128, :])
            nc.vector.tensor_mul(yT[p][0:64, :], py_a[0:64, :], rec_a[:])
            nc.vector.tensor_mul(yT[p][64:128, :], py_b[0:64, :], rec_b[:])
        while pulled < len(units):
            units[pulled]()
            pulled += 1

    # weights are loaded once per kernel invocation, outside the timing
    # repeat loop (matches the baseline measurement methodology)
    load_weights()

    def whole_body():
        state.clear()
        load_xT(0)
        # minimal phase-1 prefix for pair 0's attention; v units first
        # (wv arrives on the ACT queue before wqk finishes on sync), the
        # rest of strip 0's qk units drip into phase2(0) as filler.
        for u in v_units(0) + qk_units(0, [0]):
            u()
        for s in range(NSTRIP):
            units = []
            if s == 0:
                units.extend(qk_units(0, [1, 2, 3]))
            if s + 1 < NSTRIP:
                load_xT(s + 1)
                units.extend(qk_units(s + 1, range(NP)))
                units.extend(v_units(s + 1))
            if s == 2:
                units.extend(p3_units(0))
            if s == 3:
                units.extend(p3_units(1))
                units.extend(p3_units(2))
            phase2(s, units)
        for u in p3_units(NSTRIP - 1):
            u()

    repeat = int(os.environ.get("KREPEAT", "1"))
    if repeat > 1:
        # timing-only mode: run the whole computation `repeat` times
        # (idempotent) so marginal wall-clock per iteration = HW exec time
        with tc.For_i(0, repeat, 1):
            whole_body()
    else:
        whole_body()


_CACHE = {}


def build_nc():
    if "nc" in _CACHE:
        return _CACHE["nc"]
    nc = bacc.Bacc("TRN2", target_bir_lowering=False, debug=False,
                   enable_asserts=False, num_devices=8)
    x = nc.dram_tensor("x", [S, E], F16, kind="ExternalInput").ap()
    w_qkv = nc.dram_tensor("w_qkv", [E, 1536], F16,
                           kind="ExternalInput").ap()
    w_proj = nc.dram_tensor("w_proj", [512, E], F16,
                            kind="ExternalInput").ap()
    out = nc.dram_tensor("out", [S, E], F32, kind="ExternalOutput").ap()
    with tile.TileContext(nc) as tc:
        with ExitStack() as ctx:
            emit_kernel(ctx, tc, out, x, w_qkv, w_proj)
    nc.compile()
    _CACHE["nc"] = nc
    return nc


def make_in_maps(x, w_attn, w_proj):
    x = np.asarray(x, dtype=np.float32)
    w_attn = np.asarray(w_attn, dtype=np.float32)
    w_proj = np.asarray(w_proj, dtype=np.float32)
    in_maps = []
    for c in range(8):
        b, hg = divmod(c, 2)
        lo, hi = hg * 512, (hg + 1) * 512
        wq = w_attn[:, lo:hi]
        wk = w_attn[:, 1024 + lo:1024 + hi]
        wv = w_attn[:, 2048 + lo:2048 + hi]
        wqkv = np.ascontiguousarray(
            np.concatenate([wq, wk, wv], axis=1)).astype(np.float16)
        wp = np.ascontiguousarray(w_proj[lo:hi, :]).astype(np.float16)
        in_maps.append({
            "x": np.ascontiguousarray(x[b]).astype(np.float16),
            "w_qkv": wqkv,
            "w_proj": wp,
        })
    return in_maps


def gather(results):
    parts = [results[c]["out"] for c in range(8)]
    return np.stack([parts[2 * b] + parts[2 * b + 1] for b in range(4)]).astype(
        np.float32)


def kernel(x, w_attn, w_proj):
    nc = build_nc()
    res = run_bass_kernel_spmd(nc, make_in_maps(x, w_attn, w_proj),
                               core_ids=list(range(8)))
    return gather(res.results)
